# revision 1
# baseline (speedup 1.0000x reference)
"""HebbianConv2d Trainium2 kernel.

Full-input contract: kernel(x=(16,256,56,56) f32, weight=(384,256,3,3) f32)
-> (16,384,54,54) f32.  Data-parallel over batch across 8 NeuronCores
(2 samples/core); weight and the lateral-feedback table are replicated.

Per-core pipeline (all fp32 — the winner-take-all argmax must match the
fp32 jax reference, and measured fp32r/bf16 matmul error of ~2e-4/2e-3
flips winners; hardware fp32 matmul measures ~2e-7):
  conv      y[cout_chunk(128), s(486)] += w_slice^T @ x_window  (18 mms)
  wta       chunk-merge max (DVE) -> PE transpose -> DVE reduce ->
            PE transpose back -> DRAM-roundtrip partition broadcast
            mask = (y >= colmax)  (DVE is_ge)
  lfb       out2[c,s] = sum_j G[j,c] mask[j,s]  (3 mms, G stationary:
            fp32 stationary x one-hot moving is exact on HW)
  gate      out = min(out2, 1) * y  (DVE scalar_tensor_tensor), DMA out
"""
import sys

sys.path.insert(0, "/opt/trn_rl_repo")

import numpy as np

import concourse.bass as bass
import concourse.mybir as mybir
from concourse.bass_utils import run_bass_kernel_spmd

try:
    from tile_fix import TileContextFixed
except ImportError:
    TileContextFixed = None  # defined inline below

if TileContextFixed is None:
    import concourse.tile as tile
    from concourse.vector_clock import ScopedClock, VectorClock

    MAXW = 1

    class TileContextFixed(tile.TileContext):  # noqa: F811
        """Walrus in this container rejects >1 sync-wait per instruction;
        split excess waits onto preceding same-engine nops."""

        _ws_counter = 0

        def _add_instruction(self, inst):
            si = getattr(inst, "sync_info", None)
            eng = getattr(inst, "engine", None)
            if (
                si is not None
                and si.on_wait
                and len(si.on_wait) > MAXW
                and eng is not None
                and eng != mybir.EngineType.Unassigned
            ):
                waits = list(si.on_wait)
                keep, excess = waits[:MAXW], waits[MAXW:]
                while excess:
                    chunk, excess = excess[:MAXW], excess[MAXW:]
                    TileContextFixed._ws_counter += 1
                    nop = mybir.InstNoOp(
                        name=f"{inst.name}-ws{TileContextFixed._ws_counter}",
                        engine=eng,
                        sync_info=mybir.SyncInfo(on_wait=chunk, on_update=[]),
                        bass_nofuse=True,
                    )
                    super()._add_instruction(nop)
                inst.sync_info = mybir.SyncInfo(
                    on_wait=keep, on_update=si.on_update
                )
            super()._add_instruction(inst)

        def _drain_and_barrier(self, tick_clock, wait_clock):
            vc = tick_clock.global_clock
            n = len(vc)
            for proc in range(n):
                t = vc[proc]
                if t <= 0:
                    continue
                v = [0] * n
                v[proc] = t
                nop = self.nc.sync.nop(nofuse=True)
                wait_clock.add_sem_waits(
                    nop.ins, ScopedClock({None: VectorClock(v)})
                )
            self.nc.sync.drain()
            self.nc.all_engine_barrier()
            assert self.sems is not None
            popped = self.nc._tile_sem_poison_stack.pop()
            assert popped is self._sem_poison
            self.nc.clear_and_free_semaphores(
                list(self.sems.allocated().values())
            )
            self.nc.all_engine_barrier()


# Problem constants
B, CIN, COUT, H, W, KS = 16, 256, 384, 56, 56, 3
HOUT = H - KS + 1  # 54
MAP_RADIUS = (COUT - 1) // 2  # 191
LFB_SIGMA = float(MAP_RADIUS)
N_CORES = 8
BPC = B // N_CORES  # samples per core = 2
NCIN = CIN // 128  # 2 cin chunks
NCOUT = COUT // 128  # 3 cout chunks
ROWS_PER_BLOCK = 9
NBLK = HOUT // ROWS_PER_BLOCK  # 6 blocks per sample
SBLK = ROWS_PER_BLOCK * HOUT  # 486 spatial positions per block
DT = mybir.dt.float32


def lfb_table() -> np.ndarray:
    """G[j, c] = kern[MAP_RADIUS + j - c], the valid-conv matrix of the
    Gaussian lateral-feedback kernel over the padded channel axis."""
    d = np.abs(np.arange(COUT, dtype=np.float32) - MAP_RADIUS)
    kern = np.exp(-(d.astype(np.float32) ** 2) / np.float32(2.0 * LFB_SIGMA**2))
    kern = kern.astype(np.float32)
    G = np.zeros((COUT, COUT), np.float32)
    for c in range(COUT):
        lo = MAP_RADIUS - c
        G[:, c] = kern[np.clip(np.arange(COUT) + lo, 0, COUT - 1)]
        valid = (np.arange(COUT) + lo >= 0) & (np.arange(COUT) + lo < COUT)
        G[~valid, c] = 0.0
    return G


NSCH = (SBLK + 127) // 128  # 4 s-subchunks per block for the WTA transposes


def build_nc(repeat: int = 1):
    nc = bass.Bass()
    x = nc.declare_dram_parameter("x", [BPC, CIN, H, W], DT, isOutput=False)
    w = nc.declare_dram_parameter("w", [CIN, KS, KS, COUT], DT, isOutput=False)
    g = nc.declare_dram_parameter("g", [COUT, COUT], DT, isOutput=False)
    ident = nc.declare_dram_parameter("ident", [128, 128], DT, isOutput=False)
    out = nc.declare_dram_parameter(
        "out", [BPC, COUT, HOUT, HOUT], DT, isOutput=True
    )

    with TileContextFixed(nc) as tc:
        import contextlib

        with contextlib.ExitStack() as ctx:
            consts = ctx.enter_context(tc.tile_pool(name="consts", bufs=1))
            xpool = ctx.enter_context(tc.tile_pool(name="xpool", bufs=2))
            ypool = ctx.enter_context(tc.tile_pool(name="ysb", bufs=8))
            mpool = ctx.enter_context(tc.tile_pool(name="msk", bufs=8))
            spool = ctx.enter_context(tc.tile_pool(name="scratch", bufs=2))
            gpool = ctx.enter_context(tc.tile_pool(name="gout", bufs=8))
            yps = ctx.enter_context(
                tc.tile_pool(name="yps", bufs=3, space="PSUM")
            )
            ops = ctx.enter_context(
                tc.tile_pool(name="ops", bufs=3, space="PSUM")
            )
            tps = ctx.enter_context(
                tc.tile_pool(name="tps", bufs=1, space="PSUM")
            )
            rps = ctx.enter_context(
                tc.tile_pool(name="rps", bufs=1, space="PSUM")
            )
            drp = ctx.enter_context(
                tc.tile_pool(name="drp", bufs=2, space="DRAM")
            )

            w_sb = consts.tile([128, NCIN, KS, KS, COUT], DT)
            nc.gpsimd.dma_start(
                out=w_sb[:, :, :, :, :],
                in_=w.rearrange("(c k) kh kw o -> k c kh kw o", k=128),
            )
            g_sb = consts.tile([128, NCOUT, COUT], DT)
            nc.gpsimd.dma_start(
                out=g_sb[:, :, :],
                in_=g.rearrange("(jc k) c -> k jc c", k=128),
            )
            id_sb = consts.tile([128, 128], DT)
            nc.gpsimd.dma_start(out=id_sb[:, :], in_=ident[:, :])

            for _rep in range(repeat):
                for b in range(BPC):
                    x_sb = xpool.tile([128, NCIN, H * W], DT)
                    nc.gpsimd.dma_start(
                        out=x_sb[:, :, :],
                        in_=x[b].rearrange("(c k) h w -> k c (h w)", k=128),
                    )
                    x_hw = [
                        x_sb[:, ci, :].rearrange("k (h w) -> k h w", w=W)
                        for ci in range(NCIN)
                    ]
                    for blk in range(NBLK):
                        oh0 = blk * ROWS_PER_BLOCK
                        y_ps = []
                        for cc in range(NCOUT):
                            acc = yps.tile([128, SBLK], DT, tag="ypsum")
                            first = True
                            for ci in range(NCIN):
                                for kh in range(KS):
                                    for kw in range(KS):
                                        xwin = x_hw[ci][
                                            :,
                                            oh0 + kh : oh0 + kh + ROWS_PER_BLOCK,
                                            kw : kw + HOUT,
                                        ]
                                        nc.tensor.matmul(
                                            out=acc[:, :],
                                            lhsT=w_sb[
                                                :,
                                                ci,
                                                kh,
                                                kw,
                                                cc * 128 : (cc + 1) * 128,
                                            ],
                                            rhs=xwin,
                                            start=first,
                                            stop=(
                                                ci == NCIN - 1
                                                and kh == KS - 1
                                                and kw == KS - 1
                                            ),
                                        )
                                        first = False
                            y_ps.append(acc)

                        # y chunks PSUM -> SBUF (scalar engine; DVE is busier)
                        y_sb = []
                        for cc in range(NCOUT):
                            ysb = ypool.tile([128, SBLK], DT, tag="ysb")
                            nc.scalar.copy(out=ysb[:, :], in_=y_ps[cc][:, :])
                            y_sb.append(ysb)

                        # column max across all 384 channels:
                        # chunk-merge on DVE, then per-column max over the
                        # 128 partitions via PE transpose + DVE reduce
                        mx = spool.tile([128, SBLK], DT, tag="mx")
                        nc.vector.tensor_tensor(
                            out=mx[:, :],
                            in0=y_sb[0][:, :],
                            in1=y_sb[1][:, :],
                            op=mybir.AluOpType.max,
                        )
                        nc.vector.tensor_tensor(
                            out=mx[:, :],
                            in0=mx[:, :],
                            in1=y_sb[2][:, :],
                            op=mybir.AluOpType.max,
                        )
                        mxT = tps.tile([128, NSCH, 128], DT, tag="mxT")
                        for k in range(NSCH):
                            w_cols = min(128, SBLK - k * 128)
                            nc.tensor.transpose(
                                out=mxT[:w_cols, k, :],
                                in_=mx[:, k * 128 : k * 128 + w_cols],
                                identity=id_sb[:, :],
                            )
                        cmaxT = spool.tile([128, NSCH], DT, tag="cmaxT")
                        for k in range(NSCH):
                            nc.vector.tensor_reduce(
                                out=cmaxT[:, k : k + 1],
                                in_=mxT[:, k, :],
                                axis=mybir.AxisListType.X,
                                op=mybir.AluOpType.max,
                            )
                        rowps = rps.tile([1, NSCH, 128], DT, tag="rowps")
                        for k in range(NSCH):
                            nc.tensor.transpose(
                                out=rowps[0:1, k, :],
                                in_=cmaxT[:, k : k + 1],
                                identity=id_sb[:, :],
                            )
                        row_sb = spool.tile([1, NSCH * 128], DT, tag="rowsb")
                        nc.scalar.copy(out=row_sb[0:1, :], in_=rowps[0:1, :, :])
                        rowd = drp.tile([NSCH * 128], DT, tag="rowd")
                        nc.gpsimd.dma_start(out=rowd[:], in_=row_sb[0:1, :])
                        mx_bc = spool.tile([128, SBLK], DT, tag="mxbc")
                        rap = rowd[:]
                        nc.gpsimd.dma_start(
                            out=mx_bc[:, :],
                            in_=bass.AP(
                                tensor=rap.tensor,
                                offset=rap.offset,
                                ap=[[0, 128], [1, SBLK]],
                            ),
                        )

                        # winner mask, lateral feedback, gate, store
                        o2_ps = []
                        masks = []
                        for cc in range(NCOUT):
                            msk = mpool.tile([128, SBLK], DT, tag="mask")
                            nc.vector.tensor_tensor(
                                out=msk[:, :],
                                in0=y_sb[cc][:, :],
                                in1=mx_bc[:, :],
                                op=mybir.AluOpType.is_ge,
                            )
                            masks.append(msk)
                        for cc in range(NCOUT):
                            o2 = ops.tile([128, SBLK], DT, tag="o2psum")
                            for jc in range(NCOUT):
                                nc.tensor.matmul(
                                    out=o2[:, :],
                                    lhsT=g_sb[
                                        :, jc, cc * 128 : (cc + 1) * 128
                                    ],
                                    rhs=masks[jc][:, :],
                                    start=(jc == 0),
                                    stop=(jc == NCOUT - 1),
                                )
                            o2_ps.append(o2)
                        for cc in range(NCOUT):
                            go = gpool.tile([128, SBLK], DT, tag="gout")
                            nc.vector.scalar_tensor_tensor(
                                out=go[:, :],
                                in0=o2_ps[cc][:, :],
                                scalar=1.0,
                                in1=y_sb[cc][:, :],
                                op0=mybir.AluOpType.min,
                                op1=mybir.AluOpType.mult,
                            )
                            nc.gpsimd.dma_start(
                                out=out[
                                    b,
                                    cc * 128 : (cc + 1) * 128,
                                    oh0 : oh0 + ROWS_PER_BLOCK,
                                    :,
                                ],
                                in_=go[:, :],
                            )
    return nc


_NC_CACHE = {}


def _get_nc(repeat: int = 1):
    if repeat not in _NC_CACHE:
        _NC_CACHE[repeat] = build_nc(repeat)
    return _NC_CACHE[repeat]


def run_sharded(x, weight, repeat: int = 1):
    nc = _get_nc(repeat)
    x = np.ascontiguousarray(np.asarray(x), dtype=np.float32)
    weight = np.ascontiguousarray(np.asarray(weight), dtype=np.float32)
    w_t = np.ascontiguousarray(weight.transpose(1, 2, 3, 0))
    G = lfb_table()
    eye = np.eye(128, dtype=np.float32)
    in_maps = [
        {
            "x": np.ascontiguousarray(x[i * BPC : (i + 1) * BPC]),
            "w": w_t,
            "g": G,
            "ident": eye,
        }
        for i in range(N_CORES)
    ]
    res = run_bass_kernel_spmd(nc, in_maps, list(range(N_CORES)))
    out = np.concatenate([res.results[i]["out"] for i in range(N_CORES)], axis=0)
    return out


def kernel(x, weight):
    return run_sharded(x, weight, repeat=1)

